# revision 2
# baseline (speedup 1.0000x reference)
"""Trainium2 Bass kernel for nn_CometBertECTagging (B=64, L=512, HB=768, HC=1024, NL=7).

Reference computation (per batch row i):
  pos  = cumsum(valid[i]) - 1
  valid_output[i, pos[j]] = bert[i, j]  if valid[i, j] == 1; other slots zero
  logits[i] = concat([valid_output[i], comet[i]], -1) @ W + b

Device algorithm (data-parallel over batch, 8 rows per core):
  - logits = compact(bert @ Wb) + comet @ Wc + b: compaction applied to the
    per-token bert logits [L, 7] instead of the bert activations [L, 768].
  - compaction as matmul: S[j, slot] = (valid[j]*cumsum[j]-1 == slot), built on
    DVE via is_equal against an iota row; compacted_logits^T accumulates into
    the same PSUM tile as the comet logits (fp32r, near-fp32 precision).
  - cumsum(valid) via matmul with an upper-triangular ones matrix (exact).
  - activations cast f32->bf16 during the HBM load (SWDGE cast; HBM bytes
    unchanged), transposed h-tile-wise by normal-mode identity matmuls
    (bf16 weights -> FWL fast weight load, HAM stays warm), evicted
    PSUM->SBUF by DVE (bert) / ACT (comet), then bf16 classifier matmuls with
    W tiles stationary contract over h into [7, L] PSUM.
  - output stored transposed [7, L] per row (2KB/partition DMA writes); host
    untransposes during unshard. Tiny-chunk DMAs (<512B/partition) are
    avoided everywhere: they scramble partitions on this DMA path.
"""

import numpy as np

import concourse.bacc as bacc
import concourse.mybir as mybir
from concourse.tile import TileContext
from concourse.bass_utils import run_bass_kernel_spmd

F32 = mybir.dt.float32
F32R = mybir.dt.float32r
BF16 = mybir.dt.bfloat16
I32 = mybir.dt.int32

B, L, HB, HC, NL = 64, 512, 768, 1024, 7
NCORES = 8
RPC = B // NCORES  # batch rows per core
JT = L // 128      # j tiles per row
HTB = HB // 128    # bert h tiles
HTC = HC // 128    # comet h tiles

_PROGRAM = None


def build_program(loop_iters=None):
    nc = bacc.Bacc(target_bir_lowering=False)

    bert = nc.dram_tensor("bert", [RPC, L, HB], F32, kind="ExternalInput")
    comet = nc.dram_tensor("comet", [RPC, L, HC], F32, kind="ExternalInput")
    validn = nc.dram_tensor("validn", [128, L], I32, kind="ExternalInput")
    wtile = nc.dram_tensor("wtile", [128, 128], F32, kind="ExternalInput")
    biasr = nc.dram_tensor("biasr", [NL, L], F32, kind="ExternalInput")
    out = nc.dram_tensor("out", [RPC, NL, L], F32, kind="ExternalOutput")

    iota_np = np.broadcast_to(np.arange(L, dtype=np.float32), (128, L)).copy()
    triu_np = np.triu(np.ones((L, L), dtype=np.float32))  # T[j, j'] = 1 iff j <= j'
    ident_np = np.eye(128, dtype=np.float32)
    iota_c = nc.inline_tensor(iota_np, name="iota_c")
    triu_c = nc.inline_tensor(triu_np, name="triu_c")
    ident_c = nc.inline_tensor(ident_np, name="ident_c")

    with TileContext(nc) as tc:
        with (
            tc.tile_pool(name="const", bufs=1) as cpool,
            tc.tile_pool(name="nat", bufs=2) as npool,
            tc.tile_pool(name="txp", bufs=6) as tpool,
            tc.tile_pool(name="sel", bufs=2) as sel_pool,
            tc.tile_pool(name="small", bufs=2) as smpool,
            tc.tile_pool(name="ps_t", bufs=3, space="PSUM") as pt_pool,
            tc.tile_pool(name="ps_l", bufs=2, space="PSUM") as pl_pool,
            tc.tile_pool(name="ps_s", bufs=1, space="PSUM") as ps_pool,
        ):

            def body():
                # ---- constants / setup ----
                iota_sb = cpool.tile([128, L], F32, name="iota_sb")
                nc.sync.dma_start(out=iota_sb[:], in_=iota_c[:])
                tri_sb = cpool.tile([128, JT * L], F32R, name="tri_sb")
                for a in range(JT):
                    nc.sync.dma_start(
                        out=tri_sb[:, a * L : (a + 1) * L],
                        in_=triu_c[a * 128 : (a + 1) * 128, :].bitcast(F32R),
                    )
                ident32_sb = cpool.tile([128, 128], F32, name="ident32_sb")
                nc.sync.dma_start(out=ident32_sb[:], in_=ident_c[:])
                identb_sb = cpool.tile([128, 128], BF16, name="identb_sb")
                nc.vector.tensor_copy(out=identb_sb[:], in_=ident32_sb[:])
                w32_sb = cpool.tile([128, 128], F32, name="w32_sb")
                nc.sync.dma_start(out=w32_sb[:], in_=wtile[:])
                w_sb = cpool.tile([128, 128], BF16, name="w_sb")
                nc.vector.tensor_copy(out=w_sb[:], in_=w32_sb[:])
                bias_sb = cpool.tile([NL, L], F32, name="bias_sb")
                nc.sync.dma_start(out=bias_sb[:], in_=biasr[:])
                vrawn_sb = cpool.tile([128, L], I32, name="vrawn_sb")
                nc.sync.dma_start(out=vrawn_sb[:], in_=validn[:])
                vfn_sb = cpool.tile([128, L], F32, name="vfn_sb")
                nc.vector.tensor_copy(out=vfn_sb[:], in_=vrawn_sb[:])
                vf0_sb = cpool.tile([128, JT * RPC], F32, name="vf0_sb")
                for a in range(JT):
                    vt_ps = pt_pool.tile([128, 128], F32, tag="ps_t", name="vt_ps")
                    nc.tensor.matmul(
                        out=vt_ps[:],
                        lhsT=vfn_sb[:, a * 128 : (a + 1) * 128],
                        rhs=ident32_sb[:],
                        is_transpose=True,
                        start=True,
                        stop=True,
                    )
                    nc.vector.tensor_copy(
                        out=vf0_sb[:, a * RPC : (a + 1) * RPC], in_=vt_ps[:, :RPC]
                    )
                vf_sb = cpool.tile([128, JT * RPC], F32R, name="vf_sb")
                nc.vector.tensor_copy(out=vf_sb[:], in_=vf0_sb[:])

                # cumsum over L per row: C[j', r] = sum_{j<=j'} valid[j, r]
                cs_ps = ps_pool.tile([128, JT * RPC], F32, tag="ps_s", name="cs_ps")
                n_mm = sum(kt + 1 for kt in range(JT))
                i_mm = 0
                for mt in range(JT):
                    for kt in range(mt + 1):
                        nc.tensor.matmul(
                            out=cs_ps[:, mt * RPC : (mt + 1) * RPC],
                            lhsT=tri_sb[:, kt * L + mt * 128 : kt * L + (mt + 1) * 128],
                            rhs=vf_sb[:, kt * RPC : (kt + 1) * RPC],
                            start=(i_mm == 0),
                            stop=(i_mm == n_mm - 1),
                        )
                        i_mm += 1
                mval_sb = cpool.tile([128, JT * RPC], F32, name="mval_sb")
                nc.vector.tensor_mul(out=mval_sb[:], in0=vf0_sb[:], in1=cs_ps[:])
                nc.vector.tensor_scalar_add(mval_sb[:], mval_sb[:], -1.0)

                # ---- per-row pipeline ----
                for r in range(RPC):
                    # bert: SWDGE cast load straight to bf16
                    nat_b = npool.tile([128, JT * HB], BF16, tag="nat_b", name="nat_b")
                    for t in range(JT):
                        nc.gpsimd.dma_start(
                            out=nat_b[:, t * HB : (t + 1) * HB],
                            in_=bert[r, t * 128 : (t + 1) * 128, :],
                        )
                    # comet: HWDGE f32 load + DVE/ACT casts to bf16
                    n32_c = npool.tile([128, JT * HC], F32, tag="n32_c", name="n32_c")
                    for t in range(JT):
                        nc.sync.dma_start(
                            out=n32_c[:, t * HC : (t + 1) * HC],
                            in_=comet[r, t * 128 : (t + 1) * 128, :],
                        )
                    nat_c = npool.tile([128, JT * HC], BF16, tag="nat_c", name="nat_c")
                    for t in range(JT):
                        if t % 2 == 0:
                            nc.vector.tensor_copy(
                                out=nat_c[:, t * HC : (t + 1) * HC],
                                in_=n32_c[:, t * HC : (t + 1) * HC],
                            )
                        else:
                            nc.scalar.copy(
                                out=nat_c[:, t * HC : (t + 1) * HC],
                                in_=n32_c[:, t * HC : (t + 1) * HC],
                            )

                    # selection matrix tiles S[j_local, slot] per j-tile
                    s_tiles = []
                    for jt in range(JT):
                        s_t = sel_pool.tile([128, L], F32R, tag=f"s{jt}", name="s_t")
                        nc.vector.tensor_scalar(
                            out=s_t[:],
                            in0=iota_sb[:],
                            scalar1=mval_sb[:, jt * RPC + r : jt * RPC + r + 1],
                            scalar2=None,
                            op0=mybir.AluOpType.is_equal,
                        )
                        s_tiles.append(s_t)

                    # bert: transpose h-tiles (identity matmul), classifier
                    psum_bl = pl_pool.tile([NL, L], F32, tag="ps_bl", name="psum_bl")
                    for ht in range(HTB):
                        pt = pt_pool.tile([128, L], F32, tag="ps_t", name="pt")
                        for jt in range(JT):
                            nc.tensor.matmul(
                                out=pt[:, jt * 128 : (jt + 1) * 128],
                                lhsT=nat_b[:, jt * HB + ht * 128 : jt * HB + (ht + 1) * 128],
                                rhs=identb_sb[:],
                                start=(jt == 0),
                                stop=(jt == JT - 1),
                            )
                        tb = tpool.tile([128, L], BF16, tag="txp", name="tb")
                        nc.vector.tensor_copy(out=tb[:], in_=pt[:])
                        nc.tensor.matmul(
                            out=psum_bl[:],
                            lhsT=w_sb[:, ht * NL : (ht + 1) * NL],
                            rhs=tb[:],
                            start=(ht == 0),
                            stop=(ht == HTB - 1),
                        )

                    # comet: transpose h-tiles, classifier into psum_fin
                    psum_fin = pl_pool.tile([NL, L], F32, tag="ps_fin", name="psum_fin")
                    for ht in range(HTC):
                        pt = pt_pool.tile([128, L], F32, tag="ps_t", name="pt")
                        for jt in range(JT):
                            nc.tensor.matmul(
                                out=pt[:, jt * 128 : (jt + 1) * 128],
                                lhsT=nat_c[:, jt * HC + ht * 128 : jt * HC + (ht + 1) * 128],
                                rhs=identb_sb[:],
                                start=(jt == 0),
                                stop=(jt == JT - 1),
                            )
                        tcm = tpool.tile([128, L], BF16, tag="txp", name="tcm")
                        nc.scalar.copy(out=tcm[:], in_=pt[:])
                        nc.tensor.matmul(
                            out=psum_fin[:],
                            lhsT=w_sb[:, (HTB + ht) * NL : (HTB + ht + 1) * NL],
                            rhs=tcm[:],
                            start=(ht == 0),
                            stop=False,
                        )

                    # bert logits: evict, transpose to [j, 7], scatter via S
                    bl_sb = smpool.tile([NL, L], F32, tag="bl", name="bl_sb")
                    nc.scalar.copy(out=bl_sb[:], in_=psum_bl[:])
                    blt_ps = ps_pool.tile([128, JT * NL], F32, tag="ps_s", name="blt_ps")
                    for jt in range(JT):
                        nc.tensor.matmul(
                            out=blt_ps[:, jt * NL : (jt + 1) * NL],
                            lhsT=bl_sb[:, jt * 128 : (jt + 1) * 128],
                            rhs=ident32_sb[:NL, :NL],
                            is_transpose=True,
                            start=(jt == 0),
                            stop=(jt == JT - 1),
                        )
                    blt_sb = smpool.tile([128, JT * NL], F32R, tag="blt", name="blt_sb")
                    nc.vector.tensor_copy(out=blt_sb[:], in_=blt_ps[:])
                    for jt in range(JT):
                        nc.tensor.matmul(
                            out=psum_fin[:],
                            lhsT=blt_sb[:, jt * NL : (jt + 1) * NL],
                            rhs=s_tiles[jt][:],
                            start=False,
                            stop=(jt == JT - 1),
                        )

                    # bias add; store transposed [7, L] (host untransposes)
                    fin_sb = smpool.tile([NL, L], F32, tag="fin", name="fin_sb")
                    nc.scalar.add(fin_sb[:], psum_fin[:], bias_sb[:, 0:1])
                    nc.sync.dma_start(out=out[r], in_=fin_sb[:])

            if loop_iters is None:
                body()
            else:
                with tc.For_i(0, loop_iters, 1):
                    body()

    nc.compile()
    return nc


def build_baseline_program(loop_iters=None):
    nc = bacc.Bacc(target_bir_lowering=False)
    nc.dram_tensor("bert", [RPC, L, HB], F32, kind="ExternalInput")
    nc.dram_tensor("comet", [RPC, L, HC], F32, kind="ExternalInput")
    nc.dram_tensor("validn", [128, L], I32, kind="ExternalInput")
    nc.dram_tensor("wtile", [128, 128], F32, kind="ExternalInput")
    biasr = nc.dram_tensor("biasr", [NL, L], F32, kind="ExternalInput")
    out = nc.dram_tensor("out", [RPC, NL, L], F32, kind="ExternalOutput")
    with TileContext(nc) as tc:
        with tc.tile_pool(name="sb", bufs=2) as pool:

            def body():
                t = pool.tile([NL, L], F32)
                nc.sync.dma_start(out=t[:], in_=biasr[:])
                for r in range(RPC):
                    nc.sync.dma_start(out=out[r], in_=t[:])

            if loop_iters is None:
                body()
            else:
                with tc.For_i(0, loop_iters, 1):
                    body()
    nc.compile()
    return nc


def get_program():
    global _PROGRAM
    if _PROGRAM is None:
        _PROGRAM = build_program()
    return _PROGRAM


def make_in_maps(bert, comet, valid, w, b):
    bert = np.ascontiguousarray(np.asarray(bert, dtype=np.float32))
    comet = np.ascontiguousarray(np.asarray(comet, dtype=np.float32))
    valid = np.asarray(valid, dtype=np.int32)
    w = np.ascontiguousarray(np.asarray(w, dtype=np.float32))
    b = np.asarray(b, dtype=np.float32).reshape(NL, 1)
    b_rep = np.ascontiguousarray(np.broadcast_to(b, (NL, L)))
    w_tiled = np.zeros((128, 128), dtype=np.float32)
    w_tiled[:, : (HB + HC) // 128 * NL] = (
        w.reshape((HB + HC) // 128, 128, NL).transpose(1, 0, 2).reshape(128, -1)
    )
    in_maps = []
    for c in range(NCORES):
        rows = slice(c * RPC, (c + 1) * RPC)
        in_maps.append(
            {
                "bert": np.ascontiguousarray(bert[rows]),
                "comet": np.ascontiguousarray(comet[rows]),
                "validn": np.concatenate(
                    [valid[rows], np.zeros((128 - RPC, L), np.int32)], axis=0
                ),
                "wtile": w_tiled,
                "biasr": b_rep,
            }
        )
    return in_maps


def kernel(
    bert_sequence_output, comet_sequence_output, valid_ids, classifier_w, classifier_b
):
    nc = get_program()
    in_maps = make_in_maps(
        bert_sequence_output, comet_sequence_output, valid_ids, classifier_w, classifier_b
    )
    res = run_bass_kernel_spmd(nc, in_maps, list(range(NCORES)))
    return np.concatenate(
        [res.results[c]["out"].transpose(0, 2, 1) for c in range(NCORES)], axis=0
    )


if __name__ == "__main__":
    rng = np.random.default_rng(0)
    ins = {
        "bert_sequence_output": rng.standard_normal((B, L, HB), dtype=np.float32),
        "comet_sequence_output": rng.standard_normal((B, L, HC), dtype=np.float32),
        "valid_ids": rng.integers(0, 2, size=(B, L), dtype=np.int32),
        "classifier_w": (rng.standard_normal((HB + HC, NL)) * 0.02).astype(np.float32),
        "classifier_b": (rng.standard_normal((NL,)) * 0.02).astype(np.float32),
    }
    got = kernel(**ins)
    print("kernel output:", got.shape, got.dtype)



# revision 3
# speedup vs baseline: 1.2289x; 1.2289x over previous
"""Trainium2 Bass kernel v2 for nn_CometBertECTagging (B=64, L=512, HB=768, HC=1024, NL=7).

Reference computation (per batch row i):
  pos  = cumsum(valid[i]) - 1
  valid_output[i, pos[j]] = bert[i, j]  if valid[i, j] == 1; other slots zero
  logits[i] = concat([valid_output[i], comet[i]], -1) @ W + b

Device algorithm (data-parallel over batch, 8 rows per core):
  - logits = compact(bert @ Wb) + comet @ Wc + b: compaction applied to the
    per-token bert logits [L, 7] instead of the bert activations [L, 768].
  - host supplies activations pre-transposed to the SBUF tile layout
    [128, n_h_tiles*L] (h on partitions), so the classifier matmuls run
    directly on the loaded tiles: no PE transposes, no PSUM round-trips.
  - activations cast f32->bf16 during the HBM load (SWDGE cast; HBM bytes
    unchanged), one DMA per tensor per row (1.5-2.1 MB transfers).
  - compaction as matmul: S[j, slot] = (valid[j]*cumsum[j]-1 == slot), built on
    DVE via is_equal against an iota row; compacted bert logits accumulate into
    the same PSUM tile as the comet logits (f32r scatter matmuls).
  - cumsum(valid) via matmul with upper-triangular blocks (exact); only two
    distinct 128x128 constant blocks (all-ones and triu) are needed.
  - output stored transposed [7, L] per row (2KB/partition DMA writes); host
    untransposes during unshard. Tiny-chunk DMAs (<512B/partition) avoided.
"""

import numpy as np

import concourse.bacc as bacc
import concourse.mybir as mybir
from concourse.tile import TileContext
from concourse.bass_utils import run_bass_kernel_spmd

F32 = mybir.dt.float32
F32R = mybir.dt.float32r
BF16 = mybir.dt.bfloat16
I32 = mybir.dt.int32

B, L, HB, HC, NL = 64, 512, 768, 1024, 7
NCORES = 8
RPC = B // NCORES  # batch rows per core
JT = L // 128      # j tiles per row
HTB = HB // 128    # bert h tiles
HTC = HC // 128    # comet h tiles

_PROGRAMS = {}


def build_program(loop_iters=None):
    nc = bacc.Bacc(target_bir_lowering=False)

    bertT = nc.dram_tensor("bertT", [RPC, 128, HTB * L], F32, kind="ExternalInput")
    cometT = nc.dram_tensor("cometT", [RPC, 128, HTC * L], F32, kind="ExternalInput")
    validn = nc.dram_tensor("validn", [128, L], I32, kind="ExternalInput")
    wtile = nc.dram_tensor("wtile", [128, 128], F32, kind="ExternalInput")
    biasr = nc.dram_tensor("biasr", [NL, L], F32, kind="ExternalInput")
    out = nc.dram_tensor("out", [RPC, NL, L], F32, kind="ExternalOutput")

    iota_np = np.broadcast_to(np.arange(L, dtype=np.float32), (128, L)).copy()
    triu_np = np.triu(np.ones((128, 128), dtype=np.float32))
    ones_np = np.ones((128, 128), dtype=np.float32)
    ident_np = np.eye(128, dtype=np.float32)
    iota_c = nc.inline_tensor(iota_np, name="iota_c")
    triu_c = nc.inline_tensor(triu_np, name="triu_c")
    ones_c = nc.inline_tensor(ones_np, name="ones_c")
    ident_c = nc.inline_tensor(ident_np, name="ident_c")

    with TileContext(nc) as tc:
        with (
            tc.tile_pool(name="const", bufs=1) as cpool,
            tc.tile_pool(name="acts", bufs=1) as apool,
            tc.tile_pool(name="sel", bufs=2) as sel_pool,
            tc.tile_pool(name="small", bufs=2) as smpool,
            tc.tile_pool(name="ps_t", bufs=2, space="PSUM") as pt_pool,
            tc.tile_pool(name="ps_l", bufs=2, space="PSUM") as pl_pool,
            tc.tile_pool(name="ps_s", bufs=1, space="PSUM") as ps_pool,
        ):

            def body_full():
                # ---- activation loads: SWDGE f32->bf16 cast, 1 DMA/row/tensor
                xb = apool.tile([128, RPC * HTB * L], BF16, name="xb")
                xc = apool.tile([128, RPC * HTC * L], BF16, name="xc")
                for r in range(RPC):
                    nc.gpsimd.dma_start(
                        out=xb[:, r * HTB * L : (r + 1) * HTB * L], in_=bertT[r]
                    )
                    nc.gpsimd.dma_start(
                        out=xc[:, r * HTC * L : (r + 1) * HTC * L], in_=cometT[r]
                    )

                # ---- constants / setup ----
                iota_sb = cpool.tile([128, L], F32, name="iota_sb")
                nc.sync.dma_start(out=iota_sb[:], in_=iota_c[:])
                tri_sb = cpool.tile([128, 128], F32R, name="tri_sb")
                nc.sync.dma_start(out=tri_sb[:], in_=triu_c[:].bitcast(F32R))
                ones_sb = cpool.tile([128, 128], F32R, name="ones_sb")
                nc.sync.dma_start(out=ones_sb[:], in_=ones_c[:].bitcast(F32R))
                ident32_sb = cpool.tile([128, 128], F32, name="ident32_sb")
                nc.sync.dma_start(out=ident32_sb[:], in_=ident_c[:])
                w32_sb = cpool.tile([128, 128], F32, name="w32_sb")
                nc.sync.dma_start(out=w32_sb[:], in_=wtile[:])
                w_sb = cpool.tile([128, 128], BF16, name="w_sb")
                nc.vector.tensor_copy(out=w_sb[:], in_=w32_sb[:])
                bias_sb = cpool.tile([NL, L], F32, name="bias_sb")
                nc.sync.dma_start(out=bias_sb[:], in_=biasr[:])
                vrawn_sb = cpool.tile([128, L], I32, name="vrawn_sb")
                nc.sync.dma_start(out=vrawn_sb[:], in_=validn[:])
                vfn_sb = cpool.tile([128, L], F32, name="vfn_sb")
                nc.vector.tensor_copy(out=vfn_sb[:], in_=vrawn_sb[:])

                # transpose valid to [token(tile,p), row]
                vf0_sb = cpool.tile([128, JT * RPC], F32, name="vf0_sb")
                for a in range(JT):
                    vt_ps = pt_pool.tile([128, 128], F32, tag="ps_t", name="vt_ps")
                    nc.tensor.matmul(
                        out=vt_ps[:],
                        lhsT=vfn_sb[:, a * 128 : (a + 1) * 128],
                        rhs=ident32_sb[:],
                        is_transpose=True,
                        start=True,
                        stop=True,
                    )
                    nc.vector.tensor_copy(
                        out=vf0_sb[:, a * RPC : (a + 1) * RPC], in_=vt_ps[:, :RPC]
                    )
                vf_sb = cpool.tile([128, JT * RPC], F32R, name="vf_sb")
                nc.vector.tensor_copy(out=vf_sb[:], in_=vf0_sb[:])

                # cumsum over L per row: C[j', r] = sum_{j<=j'} valid[j, r]
                cs_ps = ps_pool.tile([128, JT * RPC], F32, tag="ps_s", name="cs_ps")
                for mt in range(JT):
                    for kt in range(mt + 1):
                        nc.tensor.matmul(
                            out=cs_ps[:, mt * RPC : (mt + 1) * RPC],
                            lhsT=(tri_sb if kt == mt else ones_sb)[:],
                            rhs=vf_sb[:, kt * RPC : (kt + 1) * RPC],
                            start=(kt == 0),
                            stop=(kt == mt),
                        )
                mval_sb = cpool.tile([128, JT * RPC], F32, name="mval_sb")
                nc.vector.tensor_mul(out=mval_sb[:], in0=vf0_sb[:], in1=cs_ps[:])
                nc.vector.tensor_scalar_add(mval_sb[:], mval_sb[:], -1.0)

                # ---- per-row pipeline ----
                for r in range(RPC):
                    xbr = xb[:, r * HTB * L : (r + 1) * HTB * L]
                    xcr = xc[:, r * HTC * L : (r + 1) * HTC * L]

                    # selection matrix tiles S[j_local, slot] per j-tile
                    s_tiles = []
                    for jt in range(JT):
                        s_t = sel_pool.tile([128, L], F32R, tag=f"s{jt}", name="s_t")
                        nc.vector.tensor_scalar(
                            out=s_t[:],
                            in0=iota_sb[:],
                            scalar1=mval_sb[:, jt * RPC + r : jt * RPC + r + 1],
                            scalar2=None,
                            op0=mybir.AluOpType.is_equal,
                        )
                        s_tiles.append(s_t)

                    # bert logits^T [7, L]
                    psum_bl = pl_pool.tile([NL, L], F32, tag="ps_bl", name="psum_bl")
                    for ht in range(HTB):
                        nc.tensor.matmul(
                            out=psum_bl[:],
                            lhsT=w_sb[:, ht * NL : (ht + 1) * NL],
                            rhs=xbr[:, ht * L : (ht + 1) * L],
                            start=(ht == 0),
                            stop=(ht == HTB - 1),
                        )
                    bl_sb = smpool.tile([NL, L], F32, tag="bl", name="bl_sb")
                    nc.scalar.copy(out=bl_sb[:], in_=psum_bl[:])

                    # transpose bert logits to [token, 7]
                    blt_ps = ps_pool.tile([128, JT * NL], F32, tag="ps_s2", name="blt_ps")
                    for jt in range(JT):
                        nc.tensor.matmul(
                            out=blt_ps[:, jt * NL : (jt + 1) * NL],
                            lhsT=bl_sb[:, jt * 128 : (jt + 1) * 128],
                            rhs=ident32_sb[:NL, :NL],
                            is_transpose=True,
                            start=(jt == 0),
                            stop=(jt == JT - 1),
                        )
                    blt_sb = smpool.tile([128, JT * NL], F32R, tag="blt", name="blt_sb")
                    nc.vector.tensor_copy(out=blt_sb[:], in_=blt_ps[:])

                    # comet logits^T + scattered bert logits into one PSUM tile
                    psum_fin = pl_pool.tile([NL, L], F32, tag="ps_fin", name="psum_fin")
                    for ht in range(HTC):
                        nc.tensor.matmul(
                            out=psum_fin[:],
                            lhsT=w_sb[:, (HTB + ht) * NL : (HTB + ht + 1) * NL],
                            rhs=xcr[:, ht * L : (ht + 1) * L],
                            start=(ht == 0),
                            stop=False,
                        )
                    for jt in range(JT):
                        nc.tensor.matmul(
                            out=psum_fin[:],
                            lhsT=blt_sb[:, jt * NL : (jt + 1) * NL],
                            rhs=s_tiles[jt][:],
                            start=False,
                            stop=(jt == JT - 1),
                        )

                    # bias add; store transposed [7, L] (host untransposes)
                    fin_sb = smpool.tile([NL, L], F32, tag="fin", name="fin_sb")
                    nc.scalar.add(fin_sb[:], psum_fin[:], bias_sb[:, 0:1])
                    nc.sync.dma_start(out=out[r], in_=fin_sb[:])

            body = body_full

            if loop_iters is None:
                body()
            else:
                with tc.For_i(0, loop_iters, 1):
                    body()

    nc.compile()
    return nc


def build_baseline_program(loop_iters=None):
    nc = bacc.Bacc(target_bir_lowering=False)
    nc.dram_tensor("bertT", [RPC, 128, HTB * L], F32, kind="ExternalInput")
    nc.dram_tensor("cometT", [RPC, 128, HTC * L], F32, kind="ExternalInput")
    nc.dram_tensor("validn", [128, L], I32, kind="ExternalInput")
    nc.dram_tensor("wtile", [128, 128], F32, kind="ExternalInput")
    biasr = nc.dram_tensor("biasr", [NL, L], F32, kind="ExternalInput")
    out = nc.dram_tensor("out", [RPC, NL, L], F32, kind="ExternalOutput")
    with TileContext(nc) as tc:
        with tc.tile_pool(name="sb", bufs=2) as pool:

            def body():
                t = pool.tile([NL, L], F32)
                nc.sync.dma_start(out=t[:], in_=biasr[:])
                for r in range(RPC):
                    nc.sync.dma_start(out=out[r], in_=t[:])

            if loop_iters is None:
                body()
            else:
                with tc.For_i(0, loop_iters, 1):
                    body()
    nc.compile()
    return nc


def get_program():
    if "main" not in _PROGRAMS:
        _PROGRAMS["main"] = build_program()
    return _PROGRAMS["main"]


def make_in_maps(bert, comet, valid, w, b):
    bert = np.asarray(bert, dtype=np.float32)
    comet = np.asarray(comet, dtype=np.float32)
    valid = np.asarray(valid, dtype=np.int32)
    w = np.ascontiguousarray(np.asarray(w, dtype=np.float32))
    b = np.asarray(b, dtype=np.float32).reshape(NL, 1)
    b_rep = np.ascontiguousarray(np.broadcast_to(b, (NL, L)))
    w_tiled = np.zeros((128, 128), dtype=np.float32)
    w_tiled[:, : (HB + HC) // 128 * NL] = (
        w.reshape((HB + HC) // 128, 128, NL).transpose(1, 0, 2).reshape(128, -1)
    )
    # activations -> SBUF tile layout [128, n_h_tiles * L] per row:
    # X[p, ht*L + l] = act[l, ht*128 + p]
    bertT = (
        bert.transpose(0, 2, 1)                # [B, HB, L]
        .reshape(B, HTB, 128, L)
        .transpose(0, 2, 1, 3)                 # [B, 128, HTB, L]
        .reshape(B, 128, HTB * L)
    )
    cometT = (
        comet.transpose(0, 2, 1)
        .reshape(B, HTC, 128, L)
        .transpose(0, 2, 1, 3)
        .reshape(B, 128, HTC * L)
    )
    in_maps = []
    for c in range(NCORES):
        rows = slice(c * RPC, (c + 1) * RPC)
        in_maps.append(
            {
                "bertT": np.ascontiguousarray(bertT[rows]),
                "cometT": np.ascontiguousarray(cometT[rows]),
                "validn": np.concatenate(
                    [valid[rows], np.zeros((128 - RPC, L), np.int32)], axis=0
                ),
                "wtile": w_tiled,
                "biasr": b_rep,
            }
        )
    return in_maps


def kernel(
    bert_sequence_output, comet_sequence_output, valid_ids, classifier_w, classifier_b
):
    nc = get_program()
    in_maps = make_in_maps(
        bert_sequence_output, comet_sequence_output, valid_ids, classifier_w, classifier_b
    )
    res = run_bass_kernel_spmd(nc, in_maps, list(range(NCORES)))
    return np.concatenate(
        [res.results[c]["out"].transpose(0, 2, 1) for c in range(NCORES)], axis=0
    )


if __name__ == "__main__":
    rng = np.random.default_rng(0)
    ins = {
        "bert_sequence_output": rng.standard_normal((B, L, HB), dtype=np.float32),
        "comet_sequence_output": rng.standard_normal((B, L, HC), dtype=np.float32),
        "valid_ids": rng.integers(0, 2, size=(B, L), dtype=np.int32),
        "classifier_w": (rng.standard_normal((HB + HC, NL)) * 0.02).astype(np.float32),
        "classifier_b": (rng.standard_normal((NL,)) * 0.02).astype(np.float32),
    }
    got = kernel(**ins)
    print("kernel output:", got.shape, got.dtype)


# revision 5
# speedup vs baseline: 1.3397x; 1.0902x over previous
"""Trainium2 Bass kernel for nn_CometBertECTagging (B=64, L=512, HB=768, HC=1024, NL=7).

Reference computation (per batch row i):
  pos  = cumsum(valid[i]) - 1
  valid_output[i, pos[j]] = bert[i, j]  if valid[i, j] == 1; other slots zero
  logits[i] = concat([valid_output[i], comet[i]], -1) @ W + b

Device algorithm (data-parallel over batch, 8 rows per core; memory-bound:
each core streams 29.4 MB of f32 activations, ~82 us at the ~358 GB/s
HBM-per-core limit):
  - logits = compact(bert @ Wb) + comet @ Wc + b: compaction applied to the
    per-token bert logits [L, 7] instead of the bert activations [L, 768].
  - host supplies activations pre-transposed to the SBUF tile layout
    [128, n_h_tiles*L] (h on partitions), so the classifier matmuls run
    directly on the loaded tiles: no PE transposes, no PSUM round-trips;
    per-partition DMA chunks are 6-8 KB contiguous.
  - activations cast f32->bf16 during the HBM load (SWDGE cast; HBM bytes
    unchanged), two DMAs per tensor per row (0.8-1.1 MB transfers); all 32
    loads are queued up front and stream back-to-back while compute follows
    per-row. All 8 rows fit in SBUF as bf16 (14.7 MB).
  - compaction as matmul: S[j, slot] = (valid[j]*cumsum[j]-1 == slot), built on
    DVE via is_equal against an iota row; the scatter matmuls OPEN the final
    PSUM accumulation group and the comet matmuls close it, so the kernel tail
    after the last comet byte is just 8 matmuls + bias + store.
  - cumsum(valid) via matmul with upper-triangular blocks (exact); iota and
    the triangular/identity constants are generated on device (gpsimd iota +
    DVE compares) so only W, bias, and valid_ids are read beyond activations.
  - output stored transposed [7, L] per row (2KB/partition DMA writes); host
    untransposes during unshard. Tiny-chunk DMAs (<512B/partition) avoided.
"""

import numpy as np

import concourse.bacc as bacc
import concourse.mybir as mybir
from concourse.tile import TileContext
from concourse.bass_utils import run_bass_kernel_spmd

F32 = mybir.dt.float32
F32R = mybir.dt.float32r
BF16 = mybir.dt.bfloat16
I32 = mybir.dt.int32

B, L, HB, HC, NL = 64, 512, 768, 1024, 7
NCORES = 8
RPC = B // NCORES
JT = L // 128
HTB = HB // 128
HTC = HC // 128

_PROGRAMS = {}


def build_program(loop_iters=None, dma_split=2, dma_only=False, iota_mode="gps", masks_bf16=False):
    nc = bacc.Bacc(target_bir_lowering=False)

    bertT = nc.dram_tensor("bertT", [RPC, 128, HTB * L], F32, kind="ExternalInput")
    cometT = nc.dram_tensor("cometT", [RPC, 128, HTC * L], F32, kind="ExternalInput")
    validn = nc.dram_tensor("validn", [RPC, L], I32, kind="ExternalInput")
    wtile = nc.dram_tensor("wtile", [128, 128], F32, kind="ExternalInput")
    biasr = nc.dram_tensor("biasr", [NL, L], F32, kind="ExternalInput")
    out = nc.dram_tensor("out", [RPC, NL, L], F32, kind="ExternalOutput")


    with TileContext(nc) as tc:
        with (
            tc.tile_pool(name="const", bufs=1) as cpool,
            tc.tile_pool(name="acts", bufs=1) as apool,
            tc.tile_pool(name="sel", bufs=2) as sel_pool,
            tc.tile_pool(name="small", bufs=2) as smpool,
            tc.tile_pool(name="ps_t", bufs=2, space="PSUM") as pt_pool,
            tc.tile_pool(name="ps_l", bufs=2, space="PSUM") as pl_pool,
            tc.tile_pool(name="ps_s", bufs=1, space="PSUM") as ps_pool,
        ):

            def issue_loads():
                xb = apool.tile([128, RPC * HTB * L], BF16, name="xb")
                xc = apool.tile([128, RPC * HTC * L], BF16, name="xc")
                if dma_split == "tail":
                    splits_b = [1] * (RPC - 1) + [2]
                    splits_c = [1] * (RPC - 1) + [4]
                else:
                    splits_b = [dma_split] * RPC
                    splits_c = [dma_split] * RPC

                def load(dram, tile, r, width, splits):
                    n = width // splits
                    for s in range(splits):
                        nc.gpsimd.dma_start(
                            out=tile[:, r * width + s * n : r * width + (s + 1) * n],
                            in_=dram[r, :, s * n : (s + 1) * n],
                        )

                for r in range(RPC):
                    load(bertT, xb, r, HTB * L, splits_b[r])
                    load(cometT, xc, r, HTC * L, splits_c[r])
                return xb, xc

            def body_dma_only():
                issue_loads()
                bias_sb = cpool.tile([NL, L], F32, name="bias_sb")
                nc.sync.dma_start(out=bias_sb[:], in_=biasr[:])
                for r in range(RPC):
                    nc.sync.dma_start(out=out[r], in_=bias_sb[:])

            def body_full():
                # iota + derived constants generated on device (no HBM reads):
                # gpsimd ops precede the SWDGE load queue
                iota_i = cpool.tile([128, L], I32, name="iota_i")
                nc.gpsimd.iota(
                    out=iota_i[:], pattern=[[1, L]], base=0, channel_multiplier=0
                )
                pidx_i = cpool.tile([128, 1], I32, name="pidx_i")
                nc.gpsimd.iota(
                    out=pidx_i[:], pattern=[[1, 1]], base=0, channel_multiplier=1
                )
                iota_sb = cpool.tile([128, L], F32, name="iota_sb")
                nc.vector.tensor_copy(out=iota_sb[:], in_=iota_i[:])
                pidx_sb = cpool.tile([128, 1], F32, name="pidx_sb")
                nc.vector.tensor_copy(out=pidx_sb[:], in_=pidx_i[:])

                xb, xc = issue_loads()

                # ---- constants / setup ----
                # tri[p, c] = (c >= p); ident[p, c] = (c == p)
                tri_sb = cpool.tile([128, 128], BF16, name="tri_sb")
                nc.vector.tensor_scalar(
                    out=tri_sb[:],
                    in0=iota_sb[:, :128],
                    scalar1=pidx_sb[:],
                    scalar2=None,
                    op0=mybir.AluOpType.is_ge,
                )
                ones_sb = cpool.tile([128, 128], BF16, name="ones_sb")
                nc.vector.memset(ones_sb[:], 1.0)
                ident32_sb = cpool.tile([128, 128], F32, name="ident32_sb")
                nc.vector.tensor_scalar(
                    out=ident32_sb[:],
                    in0=iota_sb[:, :128],
                    scalar1=pidx_sb[:],
                    scalar2=None,
                    op0=mybir.AluOpType.is_equal,
                )
                w32_sb = cpool.tile([128, 128], F32, name="w32_sb")
                nc.sync.dma_start(out=w32_sb[:], in_=wtile[:])
                w_sb = cpool.tile([128, 128], BF16, name="w_sb")
                nc.vector.tensor_copy(out=w_sb[:], in_=w32_sb[:])
                bias_sb = cpool.tile([NL, L], F32, name="bias_sb")
                nc.sync.dma_start(out=bias_sb[:], in_=biasr[:])
                vrawn_sb = cpool.tile([RPC, L], I32, name="vrawn_sb")
                nc.sync.dma_start(out=vrawn_sb[:], in_=validn[:])
                vfn_sb = cpool.tile([RPC, L], F32, name="vfn_sb")
                nc.vector.tensor_copy(out=vfn_sb[:], in_=vrawn_sb[:])

                # transpose valid to [token(tile,p), row]
                vf0_sb = cpool.tile([128, JT * RPC], F32, name="vf0_sb")
                for a in range(JT):
                    vt_ps = pt_pool.tile([128, RPC], F32, tag="ps_t", name="vt_ps")
                    nc.tensor.matmul(
                        out=vt_ps[:],
                        lhsT=vfn_sb[:, a * 128 : (a + 1) * 128],
                        rhs=ident32_sb[:RPC, :RPC],
                        is_transpose=True,
                        start=True,
                        stop=True,
                    )
                    nc.vector.tensor_copy(
                        out=vf0_sb[:, a * RPC : (a + 1) * RPC], in_=vt_ps[:, :RPC]
                    )
                vf_sb = cpool.tile([128, JT * RPC], BF16, name="vf_sb")
                nc.vector.tensor_copy(out=vf_sb[:], in_=vf0_sb[:])

                # cumsum over L per row: C[j', r] = sum_{j<=j'} valid[j, r]
                cs_ps = ps_pool.tile([128, JT * RPC], F32, tag="ps_s", name="cs_ps")
                for mt in range(JT):
                    for kt in range(mt + 1):
                        nc.tensor.matmul(
                            out=cs_ps[:, mt * RPC : (mt + 1) * RPC],
                            lhsT=(tri_sb if kt == mt else ones_sb)[:],
                            rhs=vf_sb[:, kt * RPC : (kt + 1) * RPC],
                            start=(kt == 0),
                            stop=(kt == mt),
                        )
                mval_sb = cpool.tile([128, JT * RPC], F32, name="mval_sb")
                nc.vector.tensor_mul(out=mval_sb[:], in0=vf0_sb[:], in1=cs_ps[:])
                nc.vector.tensor_scalar_add(mval_sb[:], mval_sb[:], -1.0)

                # ---- per-row pipeline ----
                for r in range(RPC):
                    xbr = xb[:, r * HTB * L : (r + 1) * HTB * L]
                    xcr = xc[:, r * HTC * L : (r + 1) * HTC * L]

                    s_tiles = []
                    for jt in range(JT):
                        s_t = sel_pool.tile(
                            [128, L], BF16 if masks_bf16 else F32R, tag=f"s{jt}", name="s_t"
                        )
                        nc.vector.tensor_scalar(
                            out=s_t[:],
                            in0=iota_sb[:],
                            scalar1=mval_sb[:, jt * RPC + r : jt * RPC + r + 1],
                            scalar2=None,
                            op0=mybir.AluOpType.is_equal,
                        )
                        s_tiles.append(s_t)

                    # bert logits^T [7, L]
                    psum_bl = pl_pool.tile([NL, L], F32, tag="ps_bl", name="psum_bl")
                    for ht in range(HTB):
                        nc.tensor.matmul(
                            out=psum_bl[:],
                            lhsT=w_sb[:, ht * NL : (ht + 1) * NL],
                            rhs=xbr[:, ht * L : (ht + 1) * L],
                            start=(ht == 0),
                            stop=(ht == HTB - 1),
                        )
                    bl_sb = smpool.tile([NL, L], F32, tag="bl", name="bl_sb")
                    nc.scalar.copy(out=bl_sb[:], in_=psum_bl[:])

                    # transpose bert logits to [token, 7]
                    blt_ps = ps_pool.tile([128, JT * NL], F32, tag="ps_s2", name="blt_ps")
                    for jt in range(JT):
                        nc.tensor.matmul(
                            out=blt_ps[:, jt * NL : (jt + 1) * NL],
                            lhsT=bl_sb[:, jt * 128 : (jt + 1) * 128],
                            rhs=ident32_sb[:NL, :NL],
                            is_transpose=True,
                            start=(jt == 0),
                            stop=(jt == JT - 1),
                        )
                    blt_sb = smpool.tile(
                        [128, JT * NL], BF16 if masks_bf16 else F32R, tag="blt", name="blt_sb"
                    )
                    nc.vector.tensor_copy(out=blt_sb[:], in_=blt_ps[:])

                    # scattered bert logits + comet logits^T into one PSUM tile;
                    # scatter first: it depends only on bert (loaded earlier),
                    # so the last row's tail is just the final comet matmuls
                    psum_fin = pl_pool.tile([NL, L], F32, tag="ps_fin", name="psum_fin")
                    for jt in range(JT):
                        nc.tensor.matmul(
                            out=psum_fin[:],
                            lhsT=blt_sb[:, jt * NL : (jt + 1) * NL],
                            rhs=s_tiles[jt][:],
                            start=(jt == 0),
                            stop=False,
                        )
                    for ht in range(HTC):
                        nc.tensor.matmul(
                            out=psum_fin[:],
                            lhsT=w_sb[:, (HTB + ht) * NL : (HTB + ht + 1) * NL],
                            rhs=xcr[:, ht * L : (ht + 1) * L],
                            start=False,
                            stop=(ht == HTC - 1),
                        )

                    fin_sb = smpool.tile([NL, L], F32, tag="fin", name="fin_sb")
                    nc.scalar.add(fin_sb[:], psum_fin[:], bias_sb[:, 0:1])
                    nc.sync.dma_start(out=out[r], in_=fin_sb[:])

            body = body_dma_only if dma_only else body_full
            if loop_iters is None:
                body()
            else:
                with tc.For_i(0, loop_iters, 1):
                    body()

    nc.compile()
    return nc


def build_baseline_program(loop_iters=None):
    nc = bacc.Bacc(target_bir_lowering=False)
    nc.dram_tensor("bertT", [RPC, 128, HTB * L], F32, kind="ExternalInput")
    nc.dram_tensor("cometT", [RPC, 128, HTC * L], F32, kind="ExternalInput")
    nc.dram_tensor("validn", [RPC, L], I32, kind="ExternalInput")
    nc.dram_tensor("wtile", [128, 128], F32, kind="ExternalInput")
    biasr = nc.dram_tensor("biasr", [NL, L], F32, kind="ExternalInput")
    out = nc.dram_tensor("out", [RPC, NL, L], F32, kind="ExternalOutput")
    with TileContext(nc) as tc:
        with tc.tile_pool(name="sb", bufs=2) as pool:

            def body():
                t = pool.tile([NL, L], F32)
                nc.sync.dma_start(out=t[:], in_=biasr[:])
                for r in range(RPC):
                    nc.sync.dma_start(out=out[r], in_=t[:])

            if loop_iters is None:
                body()
            else:
                with tc.For_i(0, loop_iters, 1):
                    body()
    nc.compile()
    return nc


def get_program():
    if "main" not in _PROGRAMS:
        _PROGRAMS["main"] = build_program()
    return _PROGRAMS["main"]


def make_in_maps(bert, comet, valid, w, b):
    bert = np.asarray(bert, dtype=np.float32)
    comet = np.asarray(comet, dtype=np.float32)
    valid = np.asarray(valid, dtype=np.int32)
    w = np.ascontiguousarray(np.asarray(w, dtype=np.float32))
    b = np.asarray(b, dtype=np.float32).reshape(NL, 1)
    b_rep = np.ascontiguousarray(np.broadcast_to(b, (NL, L)))
    w_tiled = np.zeros((128, 128), dtype=np.float32)
    w_tiled[:, : (HB + HC) // 128 * NL] = (
        w.reshape((HB + HC) // 128, 128, NL).transpose(1, 0, 2).reshape(128, -1)
    )
    bertT = (
        bert.transpose(0, 2, 1)
        .reshape(B, HTB, 128, L)
        .transpose(0, 2, 1, 3)
        .reshape(B, 128, HTB * L)
    )
    cometT = (
        comet.transpose(0, 2, 1)
        .reshape(B, HTC, 128, L)
        .transpose(0, 2, 1, 3)
        .reshape(B, 128, HTC * L)
    )
    in_maps = []
    for c in range(NCORES):
        rows = slice(c * RPC, (c + 1) * RPC)
        in_maps.append(
            {
                "bertT": np.ascontiguousarray(bertT[rows]),
                "cometT": np.ascontiguousarray(cometT[rows]),
                "validn": np.ascontiguousarray(valid[rows]),
                "wtile": w_tiled,
                "biasr": b_rep,
            }
        )
    return in_maps


def kernel(
    bert_sequence_output, comet_sequence_output, valid_ids, classifier_w, classifier_b
):
    nc = get_program()
    in_maps = make_in_maps(
        bert_sequence_output, comet_sequence_output, valid_ids, classifier_w, classifier_b
    )
    res = run_bass_kernel_spmd(nc, in_maps, list(range(NCORES)))
    return np.concatenate(
        [res.results[c]["out"].transpose(0, 2, 1) for c in range(NCORES)], axis=0
    )

